# revision 52
# baseline (speedup 1.0000x reference)
"""Multi-head causal attention (B=4, S=2048, D=1024, H=16) on 8 Trainium2
NeuronCores.

Sharding: core c handles batch c//2 and head-group c%2 (8 of 16 heads).
QKV weights are column-sharded per head-group; attention runs fully local.
The output projection is computed from LOCAL context only: Wo is
row-sharded (each core takes the rows matching its 8 heads, all 1024
columns), partial outputs for all columns are accumulated in PSUM across
the 4 local head pairs, the bias is pre-added with a zero-mask so each
pair of cores contributes it exactly once, and pairwise ReduceScatters
write the bf16 output tensor directly - no AllGather of context and no
post-collective compute.

Schedule: one continuous software-pipelined stream.  The outer loop
runs over 512-query chunks c, the inner over the 4 local head pairs, so
ALL heads' context for chunk c is final right after the chunk - the
out-projection for its tokens and the ReduceScatter piece for chunk c
fire immediately and overlap later chunks' attention; only the last
chunk's piece is exposed.  QKV-projection and out-projection units are
split into half-size granules and metered into the attention group
slots (one per slot) so the PE always has ~2.5us of work per 2.1us of
ACT exp work and never starves; a readiness-gated driver forces extra
granule drains when the next attention slot still needs its kq
projection.  The ctx matmuls lag the score matmuls by one group; the
normalize stage2 lags its stage1 by one slot (reciprocal latency).
Score and ctx matmuls on diagonal blocks are narrowed to the
causally-needed query range; the remaining intra-block triangle is
zeroed post-exp with gpsimd affine_select.
"""

from collections import deque

import numpy as np

import concourse.bass as bass
import concourse.tile as tile
from concourse import bacc, mybir
from concourse.bass import ts
from concourse.bass_utils import run_bass_kernel_spmd
from concourse.masks import make_identity

B, S, D, H, HD = 4, 2048, 1024, 16, 64
P = 128
DPC = 512                 # q/k/v dims per core (8 heads)
NT = S // P               # 16 token chunks
NKO = D // P              # 8 contraction chunks of the model dim
NQ = S // 512             # 4 q chunks of 512
NHP = DPC // P            # 4 local head pairs
F32 = mybir.dt.float32
FR = mybir.dt.float32r
BF16 = mybir.dt.bfloat16
EXP = mybir.ActivationFunctionType.Exp
MUL = mybir.AluOpType.mult
ADD = mybir.AluOpType.add
GROUPS = [[0, 1], [2, 3], [4, 5], [6, 7]]

_CACHE = {}


def build_nc():
    nc = bacc.Bacc("TRN2", target_bir_lowering=False, debug=False, num_devices=8)

    x_d = nc.declare_dram_parameter("x", [S, D], BF16, isOutput=False)
    wq_d = nc.declare_dram_parameter("wq", [D, DPC], BF16, isOutput=False)
    wk_d = nc.declare_dram_parameter("wk", [D, DPC], BF16, isOutput=False)
    wv_d = nc.declare_dram_parameter("wv", [D, DPC], BF16, isOutput=False)
    wo_d = nc.declare_dram_parameter("wo", [DPC, D], BF16, isOutput=False)
    bo_d = nc.declare_dram_parameter("bo", [P, D], F32, isOutput=False)
    out_d = nc.declare_dram_parameter("out", [NT, P, DPC], BF16, isOutput=True)

    with tile.TileContext(nc) as tc:
        with (
            tc.tile_pool(name="const", bufs=1) as cst,
            tc.tile_pool(name="big", bufs=1) as big,
            tc.tile_pool(name="dram", bufs=1, space="DRAM") as dramp,
            tc.tile_pool(name="cp", bufs=1) as cp,
            tc.tile_pool(name="ep", bufs=2) as ep,
            tc.tile_pool(name="xst", bufs=3) as xst,
            tc.tile_pool(name="cxd", bufs=3) as cxd,
            tc.tile_pool(name="pos", bufs=2) as pos,
            tc.tile_pool(name="psS", bufs=1, space="PSUM") as pss,
            tc.tile_pool(name="psC", bufs=1, space="PSUM") as psc,
            tc.tile_pool(name="psX", bufs=2, space="PSUM") as psx,
        ):
            ident = cst.tile([P, P], BF16)
            make_identity(nc, ident[:])
            ones_f = cst.tile([P, 64], F32)
            nc.vector.memset(ones_f[:], 1.0)
            ones_fr = cst.tile([P, 64], FR)
            nc.vector.tensor_copy(ones_fr[:], ones_f[:])
            bo_sb = cst.tile([P, D], F32)

            # persistent intermediates
            xT = big.tile([P, NKO, S], BF16)
            qT = big.tile([P, NHP, S], BF16)       # [dh, pair, tok]
            kT = big.tile([P, NHP, S], BF16)
            v_sb = big.tile([P, NT, 8, 65], BF16)  # [tok, chunk, head, dh+1]
            wk_sb = big.tile([P, NKO, DPC], BF16)
            wq_sb = big.tile([P, NKO, DPC], BF16)
            wv_sb = big.tile([P, NKO, DPC], BF16)
            wo_sb = big.tile([P, NHP, D], BF16)    # [dh-in-pair, pair, col]
            nc.vector.memset(v_sb[:, :, :, 64:65], 1.0)

            # attention close units: the last 512-query chunk is split
            # into two 256-query subs so its RS pieces stagger and only
            # a 2-token-chunk piece remains after the final close
            SUBS = [(0, 512), (512, 512), (1024, 512), (1536, 256),
                    (1792, 256)]
            # ReduceScatter staging (DRAM), one piece per sub; the
            # collective may not write IO tensors, so it lands in an
            # internal rs_out which is DMA'd to the output parameter
            rs_in = [dramp.tile([2, w // P, P, DPC], BF16, name=f"rs_in{si}")
                     for si, (q_lo, w) in enumerate(SUBS)]
            rs_out = [dramp.tile([w // P, P, DPC], BF16, name=f"rs_out{si}")
                      for si, (q_lo, w) in enumerate(SUBS)]

            # ---- emission helpers ----
            kq_flags = set()

            v_done = [0]

            def v_unit(t):
                def emit():
                    pv = psx.tile([P, DPC], F32, tag="x", name="pv")
                    for ko in range(NKO):
                        nc.tensor.matmul(
                            pv[:],
                            xT[:, ko, ts(t, P)],
                            wv_sb[:, ko, :],
                            start=(ko == 0),
                            stop=(ko == NKO - 1),
                        )
                    nc.vector.tensor_copy(
                        v_sb[:, t, :, 0:64],
                        pv[:].rearrange("p (h d) -> p h d", h=8),
                    )
                    v_done[0] += 1
                return emit

            def proj_granules(w_sb, outT, hp, n, key):
                # one qkv projection split into two 4-matmul granules
                # sharing a single open PSUM chain
                st = {}

                def g1():
                    st["pq"] = psx.tile([P, DPC], F32, tag="x", name="pq")
                    for ko in range(4):
                        nc.tensor.matmul(
                            st["pq"][:],
                            w_sb[:, ko, ts(hp, P)],
                            xT[:, ko, ts(n, 512)],
                            start=(ko == 0),
                            stop=False,
                        )

                def g2():
                    pq = st["pq"]
                    for ko in range(4, NKO):
                        nc.tensor.matmul(
                            pq[:],
                            w_sb[:, ko, ts(hp, P)],
                            xT[:, ko, ts(n, 512)],
                            start=False,
                            stop=(ko == NKO - 1),
                        )
                    nc.vector.tensor_copy(outT[:, hp, ts(n, 512)], pq[:])
                    kq_flags.add(key)

                return [g1, g2]

            def queue_kq(hp, n):
                for fn in proj_granules(wk_sb, kT, hp, n, ("k", hp, n)):
                    pend_lo.append(fn)
                for fn in proj_granules(wq_sb, qT, hp, n, ("q", hp, n)):
                    pend_lo.append(fn)

            def queue_kq_k_first(n):
                # all k granules before any q granule: wq loads after wk,
                # and a drained granule whose weight DMA is still in
                # flight jams the 4-deep PE wait queue, head-of-line
                # blocking ready work behind it
                for hp in range(NHP):
                    for fn in proj_granules(wk_sb, kT, hp, n, ("k", hp, n)):
                        pend_lo.append(fn)
                for hp in range(NHP):
                    for fn in proj_granules(wq_sb, qT, hp, n, ("q", hp, n)):
                        pend_lo.append(fn)

            def po_granule(ctx_c, si, tq0, tq, half):
                # out-projection for token chunk t = 4c+tq, one column
                # half: contract all 4 local head pairs' ctx in one PSUM
                # chain, add the (zero-masked) bias, stage to the RS piece.
                def emit():
                    if half == 0:
                        po_sb = pos.tile([P, D], BF16, tag="po", name="po_sb")
                        po_sbs[(si, tq)] = po_sb
                    else:
                        po_sb = po_sbs.pop((si, tq))
                    pp = psx.tile([P, DPC], F32, tag="x", name="pp")
                    for hp2 in range(NHP):
                        nc.tensor.matmul(
                            pp[:],
                            ctx_c[:, hp2, ts(tq, P)],
                            wo_sb[:, hp2, ts(half, DPC)],
                            start=(hp2 == 0),
                            stop=(hp2 == NHP - 1),
                        )
                    nc.vector.tensor_tensor(
                        po_sb[:, ts(half, DPC)], pp[:],
                        bo_sb[:, ts(half, DPC)], ADD,
                    )
                    nc.sync.dma_start(
                        rs_in[si][half, tq - tq0, :, :],
                        po_sb[:, ts(half, DPC)]
                    )
                return emit

            po_sbs = {}

            def rs_unit(si):
                def emit():
                    q_lo, w = SUBS[si]
                    t0, ntc = q_lo // P, w // P
                    nc.gpsimd.collective_compute(
                        "ReduceScatter",
                        mybir.AluOpType.add,
                        replica_groups=GROUPS,
                        ins=[rs_in[si][:]],
                        outs=[rs_out[si][:]],
                    )
                return emit

            pendn = deque()
            deferred_po = []
            pend_hi = deque()   # po granules + RS units (tile-lifetime
                                # critical: must drain soon after close)
            pend_lo = deque()   # kq projection granules

            def drain(q, n):
                for _ in range(min(n, len(q))):
                    q.popleft()()

            def drain_any(n):
                for _ in range(n):
                    if pend_hi:
                        pend_hi.popleft()()
                    elif pend_lo:
                        pend_lo.popleft()()
                    else:
                        return

            def normalize(pctx, ctx_c, hp, h01, qo, w):
                # stage 1 (runs at sub close): copy the unnormalized ctx
                # to SBUF and take the reciprocal of the denominator row -
                # this frees the pctx PSUM bank for the next head pair.
                def stage1():
                    ctxu = cp.tile([64, 512], F32, tag=f"cu{hp % 2}{h01}",
                                   name="ctxu")
                    nc.vector.tensor_copy(ctxu[:, 0:w], pctx[0:64, 0:w])
                    rec = cp.tile([P, 512], FR, tag=f"rec{hp % 2}{h01}",
                                  name="rec")
                    with nc.allow_low_precision(reason="softmax recip"):
                        nc.vector.reciprocal(rec[64:65, 0:w],
                                             pctx[64:65, 0:w])

                    # stage 2: rank-1 broadcast via PE, then scale directly
                    # from PSUM (DVE reads the broadcast as an operand).
                    # The odd head lands on ctx_c partitions 64:128 via a
                    # PE identity matmul (engines cannot shift partitions).
                    def stage2():
                        pscl = psx.tile([P, DPC], F32, tag="x", name="pscl")
                        nc.tensor.matmul(
                            pscl[0:64, 0:w], ones_fr[64:65, :],
                            rec[64:65, 0:w],
                            start=True, stop=True,
                        )
                        if h01 == 0:
                            nc.vector.tensor_tensor(
                                ctx_c[0:64, hp, qo:qo + w], ctxu[:, 0:w],
                                pscl[0:64, 0:w], MUL,
                            )
                        else:
                            tmp = cp.tile([64, 512], BF16, tag="tmp",
                                          name="tmp")
                            nc.vector.tensor_tensor(
                                tmp[:, 0:w], ctxu[:, 0:w], pscl[0:64, 0:w],
                                MUL
                            )
                            psh = psx.tile([P, DPC], F32, tag="x",
                                           name="psh")
                            nc.tensor.matmul(
                                psh[64:128, 0:w], ident[0:64, 0:64],
                                tmp[:, 0:w],
                                start=True, stop=True,
                            )
                            nc.vector.tensor_copy(
                                ctx_c[64:128, hp, qo:qo + w],
                                psh[64:128, 0:w]
                            )

                    pendn.append(stage2)
                return stage1

            def attn_stream():
                """One generator over all close units (subs); yields
                (c, hp, nkb) before emitting each group slot."""
                es = {}
                prev = []
                ctx_of = {}

                def ctx_mms(q_lo, w, hp, gi, m0, nb, pctx, nkb):
                    for h01 in range(2):
                        e = es.pop((h01, gi))
                        for j in range(nb):
                            m = m0 + j
                            q0 = max(0, 128 * m - q_lo)
                            nc.tensor.matmul(
                                pctx[h01][0:65, q0:w],
                                v_sb[:, m, 2 * hp + h01, 0:65],
                                e[:, j, q0:w],
                                start=(m == 0),
                                stop=(m == nkb - 1),
                            )

                def sub_close(si, hp, pctx, ctx_c, groups):
                    q_lo, w = SUBS[si]
                    c, qo = q_lo // 512, q_lo % 512
                    nkb = (q_lo + w) // 128
                    gi = len(groups) - 1
                    ctx_mms(q_lo, w, hp, gi, groups[gi][0], groups[gi][1],
                            pctx, nkb)
                    normalize(pctx[0], ctx_c, hp, 0, qo, w)()
                    normalize(pctx[1], ctx_c, hp, 1, qo, w)()
                    if hp == NHP - 1:
                        # ctx for this sub is final once the stage2s run;
                        # emit them, then queue the out-projection granules
                        # and the RS piece.  The last sub's go out inline -
                        # they are the critical path to the final
                        # collective.
                        while pendn:
                            pendn.popleft()()
                        tq0 = qo // P
                        gr = [po_granule(ctx_c, si, tq0, tq, half)
                              for tq in range(tq0, (qo + w) // P)
                              for half in range(2)]
                        gr.append(rs_unit(si))
                        if si == len(SUBS) - 1:
                            for fn in gr:
                                fn()
                        else:
                            pend_hi.extend(gr)

                for si, (q_lo, w) in enumerate(SUBS):
                    c, qo = q_lo // 512, q_lo % 512
                    nkb = (q_lo + w) // 128
                    # block groups per slot: 2 blocks at width 512, 4 at
                    # width 256 - keeps ACT work per slot (and so the
                    # per-slot exp overhead) constant
                    gsz = 2 if w == 512 else 4
                    groups, m0 = [], 0
                    while m0 < nkb:
                        nb = min(gsz, nkb - m0)
                        groups.append((m0, nb))
                        m0 += nb
                    if qo == 0:
                        ctx_of[c] = cxd.tile([P, NHP, 512], BF16, tag="ctx",
                                             name=f"ctx{c}")
                        if 1 <= c <= NQ - 2:
                            for hp in range(NHP):
                                queue_kq(hp, c + 1)
                    ctx_c = ctx_of[c]
                    for hp in range(NHP):
                        pctx = (
                            psc.tile([P, 512], F32, tag="c0", name="pctxE"),
                            psc.tile([P, 512], F32, tag="c1", name="pctxO"),
                        )
                        for gi, (m0, nb) in enumerate(groups):
                            yield (c, hp, nkb)
                            # lagged normalize stage2s first (their
                            # reciprocals have had a full slot)
                            drain(pendn, 2)
                            sg = [
                                pss.tile([P, nb, w], F32, tag=f"s{h}",
                                         name=f"sg{h}")
                                for h in range(2)
                            ]
                            # adjacent even/odd-head matmuls with
                            # 64-partition operands; diagonal blocks
                            # narrowed to the causal query range
                            for j in range(nb):
                                m = m0 + j
                                q0 = max(0, 128 * m - q_lo)
                                for h01 in range(2):
                                    off = 64 * h01
                                    nc.tensor.matmul(
                                        sg[h01][:, j, q0:w],
                                        kT[off:off + 64, hp, ts(m, P)],
                                        qT[off:off + 64, hp,
                                           q_lo + q0:q_lo + w],
                                        start=True,
                                        stop=True,
                                    )
                            for h01 in range(2):
                                e = ep.tile([P, nb, w], BF16, tag=f"e{h01}",
                                            name=f"e{h01}")
                                es[(h01, gi)] = e
                                nc.scalar.activation(
                                    e[:], sg[h01][:], EXP, scale=0.125
                                )
                                # causal mask: zero the intra-block
                                # triangle of each diagonal 128x128 block
                                for j in range(nb):
                                    m = m0 + j
                                    if 128 * m >= q_lo:
                                        q0 = 128 * m - q_lo
                                        nc.gpsimd.affine_select(
                                            out=e[:, j, q0:q0 + 128],
                                            in_=e[:, j, q0:q0 + 128],
                                            compare_op=mybir.AluOpType.is_ge,
                                            fill=0.0,
                                            base=0,
                                            pattern=[[1, 128]],
                                            channel_multiplier=-1,
                                        )
                            # deferred ctx matmuls ride behind the feed
                            for fn in prev:
                                fn()
                            prev = []
                            drain_any(1)
                            if gi == len(groups) - 1:
                                prev.append(
                                    lambda si=si, hp=hp, pctx=pctx,
                                    ctx_c=ctx_c, groups=groups: sub_close(
                                        si, hp, pctx, ctx_c, groups))
                            else:
                                prev.append(
                                    lambda q_lo=q_lo, w=w, hp=hp, gi=gi,
                                    m0=m0, nb=nb, pctx=pctx,
                                    nkb=nkb: ctx_mms(q_lo, w, hp, gi, m0,
                                                     nb, pctx, nkb))
                for fn in prev:
                    fn()
                while pendn:
                    pendn.popleft()()

            def slot_ready(c, hp, nkb):
                if v_done[0] < nkb:
                    return False
                return ("q", hp, c) in kq_flags and ("k", hp, c) in kq_flags

            stream = attn_stream()
            cur = [None]

            def advance():
                try:
                    cur[0] = next(stream)
                except StopIteration:
                    cur[0] = False
                return cur[0]

            def pull():
                # advance the attention stream if its next slot is ready
                if cur[0] is None:
                    advance()
                if cur[0] is not False and slot_ready(*cur[0]):
                    advance()
                    return True
                return False

            # ---- phase A: transpose x; v + kq + early attention ride ----
            with nc.named_scope("phaseA"):
                for t in range(NT):
                    # weight loads metered between the x streams to match
                    # first use: wv before wk/wq (v units ride from t=3),
                    # wo/bo last (first needed by the c0 out-projection).
                    # A unit emitted before its weight DMA lands jams the
                    # 4-deep PE wait queue, so emission order here tracks
                    # the serialized DMA arrival order.
                    if t == 1:
                        for h_ in range(4):
                            nc.gpsimd.dma_start(
                                wv_sb[:, 2 * h_:2 * h_ + 2, :],
                                wv_d.rearrange("(ko p) n -> p ko n", p=P)[:, 2 * h_:2 * h_ + 2, :])
                    elif t == 4:
                        for h_ in range(4):
                            nc.gpsimd.dma_start(
                                wk_sb[:, 2 * h_:2 * h_ + 2, :],
                                wk_d.rearrange("(ko p) n -> p ko n", p=P)[:, 2 * h_:2 * h_ + 2, :])
                    elif t == 6:
                        for h_ in range(4):
                            nc.gpsimd.dma_start(
                                wq_sb[:, 2 * h_:2 * h_ + 2, :],
                                wq_d.rearrange("(ko p) n -> p ko n", p=P)[:, 2 * h_:2 * h_ + 2, :])
                    elif t == 8:
                        nc.gpsimd.dma_start(
                            wo_sb[:], wo_d.rearrange("(hp p) n -> p hp n", p=P))
                        nc.sync.dma_start(bo_sb[:], bo_d[:])
                    x_st = xst.tile([P, D], BF16, tag="x")
                    nc.sync.dma_start(
                        x_st[:], x_d.rearrange("(t p) d -> p t d", p=P)[:, t, :]
                    )
                    if t == 6:
                        queue_kq_k_first(0)
                    elif t == 12:
                        queue_kq_k_first(1)
                    if t >= 3:
                        v_unit(t - 3)()
                    if t >= 6:
                        drain_any(1 if t < 10 else 3)
                    if t >= 10:
                        pull()
                        pull()
                    for half in range(2):
                        pt = psx.tile([P, 4, P], BF16, tag="x", name="pt")
                        for q in range(4):
                            nc.tensor.transpose(
                                pt[:, q, :], x_st[:, ts(4 * half + q, P)], ident[:]
                            )
                        nc.vector.tensor_copy(
                            xT[:, 4 * half:4 * half + 4, ts(t, P)], pt[:]
                        )
                for t in range(NT - 3, NT):
                    v_unit(t)()

            # ---- main driver: attention stream + metered granules ----
            while cur[0] is not False:
                if not pull():
                    if pend_hi or pend_lo:
                        drain_any(1)
                    else:
                        # next slot not ready and nothing queued: the kq
                        # for it must already be emitted - just advance
                        advance()
            drain_any(len(pend_hi) + len(pend_lo))

            # rs_out -> output-parameter copies, all emitted last: each
            # waits its collective, and a DMA queued mid-stream would
            # head-of-line block the po staging DMAs behind it.  The
            # first four run during the last collective; only the last
            # piece's 0.7us copy lands after it.
            for si, (q_lo, w) in enumerate(SUBS):
                t0, ntc = q_lo // P, w // P
                for j in range(ntc):
                    nc.sync.dma_start(out_d[t0 + j], rs_out[si][j])

    nc.compile()
    return nc


def _bf16(a):
    import ml_dtypes
    return np.asarray(a, dtype=np.float32).astype(ml_dtypes.bfloat16)


def make_input_maps(x, Wq, Wk, Wv, Wo, bo):
    x = np.asarray(x, dtype=np.float32)
    Wq = np.asarray(Wq, dtype=np.float32)
    Wk = np.asarray(Wk, dtype=np.float32)
    Wv = np.asarray(Wv, dtype=np.float32)
    Wo = np.asarray(Wo, dtype=np.float32)
    bo = np.asarray(bo, dtype=np.float32)
    ins = []
    for c in range(8):
        b, g = c // 2, c % 2
        cols = slice(DPC * g, DPC * g + DPC)
        # bias only on this core's output columns; the peer contributes 0
        bo_t = np.zeros((P, D), np.float32)
        bo_t[:, cols] = bo[cols]
        ins.append({
            "x": _bf16(x[b]),
            "wq": _bf16(Wq[:, cols]),
            "wk": _bf16(Wk[:, cols]),
            "wv": _bf16(Wv[:, cols]),
            "wo": _bf16(Wo[cols, :]),
            "bo": bo_t,
        })
    return ins


def assemble(results):
    out = np.empty((B, S, D), np.float32)
    for c in range(8):
        b, g = c // 2, c % 2
        out[b, :, DPC * g:DPC * g + DPC] = (
            results[c]["out"].astype(np.float32).reshape(S, DPC)
        )
    return out


def kernel(x, Wq, Wk, Wv, Wo, bo):
    if "nc" not in _CACHE:
        _CACHE["nc"] = build_nc()
    nc = _CACHE["nc"]
    ins = make_input_maps(x, Wq, Wk, Wv, Wo, bo)
    res = run_bass_kernel_spmd(nc, ins, list(range(8)))
    return assemble(res.results)
